# revision 7
# baseline (speedup 1.0000x reference)
"""Trainium2 Bass kernel for nn_ConstraintDecoderModel.

Data-parallel over batch B=128 across 8 NeuronCores (16 batches/core).
All compute is formulated as PE matmuls in feature-major layout:

  per core (tokens n = b_local*64 + t, 1024 tokens):
    obj_in.T (feature-major, 24 k-tiles of 128) =
        [heads.T (host pre-transposed) ; q_e.T ; r_e.T]
    q_e.T / r_e.T   via one-hot gather matmuls  (lhsT = src_nat, rhs = [Q.T|R.T])
    ptr.T [1024,n]  = W'^T-contract over 16 k-tiles + (type-emb + bias) fold (K=5)
    logits [t,s]    = per-batch matmul  lhsT = ptr.T chunk, rhs = src.T chunk
                      + (-inf) padding-mask row via K=1 matmul
    dir/type sel    = [8, n] matmul over 24 k-tiles + fold (K=5)

Weights are replicated per core; type-embedding contributions are folded as
ET = type_emb @ W_block.T (host, exact layout transform + tiny matmul).
"""
import numpy as np

NCORES = 8
D = 1024
B = 128
S = 128
TH = 64          # real constraints per scene (= N_PER)
BPC = 16         # batches per core
NG = 4           # groups per core
GB = 4           # batches per group
NTG = GB * TH    # tokens per group = 256
NTOK = BPC * TH  # tokens per core = 1024

_NC_CACHE = None


def _build_nc():
    import concourse.bass as bass  # noqa: F401
    import concourse.mybir as mybir
    import concourse.tile as tile
    from concourse import bacc
    from contextlib import ExitStack

    F32 = mybir.dt.float32
    nc = bacc.Bacc(num_devices=NCORES)

    wmT = nc.declare_dram_parameter("wmT", [128, 8, 16, 128], F32, isOutput=False).ap()
    headsT = nc.declare_dram_parameter("headsT", [128, NG, 8, NTG], F32, isOutput=False).ap()
    src_nat = nc.declare_dram_parameter("src_nat", [128, BPC, D], F32, isOutput=False).ap()
    srcT = nc.declare_dram_parameter("srcT", [128, BPC, 8, S], F32, isOutput=False).ap()
    qr1h = nc.declare_dram_parameter("qr1h", [128, BPC, 128], F32, isOutput=False).ap()
    ty1h = nc.declare_dram_parameter("ty1h", [5, NTOK], F32, isOutput=False).ap()
    maskb = nc.declare_dram_parameter("maskb", [1, BPC, S], F32, isOutput=False).ap()
    fold_main = nc.declare_dram_parameter("fold_main", [5, D], F32, isOutput=False).ap()
    wsT = nc.declare_dram_parameter("wsT", [128, 24, 8], F32, isOutput=False).ap()
    fold_small = nc.declare_dram_parameter("fold_small", [5, 8], F32, isOutput=False).ap()

    out_obj = nc.declare_dram_parameter("out_obj", [TH, BPC, S], F32, isOutput=True).ap()
    out_small = nc.declare_dram_parameter("out_small", [8, NTOK], F32, isOutput=True).ap()

    with tile.TileContext(nc) as tc, ExitStack() as ctx:
        const = ctx.enter_context(tc.tile_pool(name="const", bufs=1))
        objp = ctx.enter_context(tc.tile_pool(name="objp", bufs=2))
        srcnp = ctx.enter_context(tc.tile_pool(name="srcnp", bufs=1))
        srctp = ctx.enter_context(tc.tile_pool(name="srctp", bufs=1))
        ptrp = ctx.enter_context(tc.tile_pool(name="ptrp", bufs=1))
        stagep = ctx.enter_context(tc.tile_pool(name="stagep", bufs=2))
        qr_ps = ctx.enter_context(tc.tile_pool(name="qr_ps", bufs=2, space="PSUM"))
        main_ps = ctx.enter_context(tc.tile_pool(name="main_ps", bufs=2, space="PSUM"))
        misc_ps = ctx.enter_context(tc.tile_pool(name="misc_ps", bufs=2, space="PSUM"))

        # ---- resident constants ----
        wm_sb = const.tile([128, 8, 16, 128], F32)
        for m in range(8):
            nc.sync.dma_start(out=wm_sb[:, m], in_=wmT[:, m])
        qr_sb = const.tile([128, BPC, 128], F32)
        nc.sync.dma_start(out=qr_sb, in_=qr1h)
        ty_sb = const.tile([5, NTOK], F32)
        nc.sync.dma_start(out=ty_sb, in_=ty1h)
        foldm_sb = const.tile([5, D], F32)
        nc.sync.dma_start(out=foldm_sb, in_=fold_main)
        ws_sb = const.tile([128, 24, 8], F32)
        nc.sync.dma_start(out=ws_sb, in_=wsT)
        folds_sb = const.tile([5, 8], F32)
        nc.sync.dma_start(out=folds_sb, in_=fold_small)
        # mask broadcast across the 64 t-partitions (DMA with partition-step 0)
        mask_bc = const.tile([TH, BPC, S], F32)
        mask_src = bass.AP(
            tensor=maskb.tensor, offset=maskb.offset,
            ap=[[0, TH], [S, BPC], [1, S]],
        )
        nc.gpsimd.dma_start(out=mask_bc, in_=mask_src)

        for g in range(NG):
            # ---- per-group input tiles ----
            objT = objp.tile([128, 24, NTG], F32)
            nc.sync.dma_start(out=objT[:, 0:8, :], in_=headsT[:, g])
            srcn = srcnp.tile([128, GB, D], F32)
            nc.sync.dma_start(out=srcn, in_=src_nat[:, g * GB:(g + 1) * GB, :])
            srct = srctp.tile([128, GB, 8, S], F32)
            nc.sync.dma_start(out=srct, in_=srcT[:, g * GB:(g + 1) * GB, :, :])

            # ---- q/r one-hot gather matmuls -> objT rows 8..23 ----
            for bl in range(GB):
                bg = g * GB + bl
                qp = qr_ps.tile([128, 8, 128], F32)
                for ch in range(8):
                    nc.tensor.matmul(
                        qp[:, ch, :],
                        lhsT=srcn[:, bl, ch * 128:(ch + 1) * 128],
                        rhs=qr_sb[:, bg, :],
                        start=True, stop=True,
                    )
                nc.any.tensor_copy(objT[:, 8:16, bl * TH:(bl + 1) * TH], qp[:, :, 0:TH])
                nc.any.tensor_copy(objT[:, 16:24, bl * TH:(bl + 1) * TH], qp[:, :, TH:128])

            # ---- main matmul: ptr.T [1024, 256] ----
            ptrT = ptrp.tile([128, 8, NTG], F32)
            for m in range(8):
                mp = main_ps.tile([128, NTG], F32)
                nc.tensor.matmul(
                    mp, lhsT=foldm_sb[:, m * 128:(m + 1) * 128],
                    rhs=ty_sb[:, g * NTG:(g + 1) * NTG],
                    start=True, stop=False,
                )
                for k in range(16):
                    nc.tensor.matmul(
                        mp, lhsT=wm_sb[:, m, k, :], rhs=objT[:, k, :],
                        start=False, stop=(k == 15),
                    )
                nc.any.tensor_copy(ptrT[:, m, :], mp)

            # ---- dir/type selections: [8, 256] ----
            sp = misc_ps.tile([8, NTG], F32, tag="misc")
            nc.tensor.matmul(
                sp, lhsT=folds_sb, rhs=ty_sb[:, g * NTG:(g + 1) * NTG],
                start=True, stop=False,
            )
            for k in range(24):
                nc.tensor.matmul(
                    sp, lhsT=ws_sb[:, k, :], rhs=objT[:, k, :],
                    start=False, stop=(k == 23),
                )
            sst = stagep.tile([8, NTG], F32)
            nc.any.tensor_copy(sst, sp)
            nc.sync.dma_start(out=out_small[:, g * NTG:(g + 1) * NTG], in_=sst)

            # ---- logits per batch: [64, 128] ----
            ost = stagep.tile([TH, GB, S], F32)
            for bl in range(GB):
                bg = g * GB + bl
                lp = misc_ps.tile([TH, S], F32, tag="misc")
                for ch in range(8):
                    nc.tensor.matmul(
                        lp,
                        lhsT=ptrT[:, ch, bl * TH:(bl + 1) * TH],
                        rhs=srct[:, bl, ch, :],
                        start=(ch == 0), stop=(ch == 7),
                    )
                nc.any.tensor_add(ost[:, bl, :], lp, mask_bc[:, bg, :])
            nc.sync.dma_start(out=out_obj[:, g * GB:(g + 1) * GB, :], in_=ost)

    nc.compile()
    return nc


def _get_nc():
    global _NC_CACHE
    if _NC_CACHE is None:
        _NC_CACHE = _build_nc()
    return _NC_CACHE


def _prep_in_maps(inputs):
    dec = np.asarray(inputs["decoded_output"], dtype=np.float32)
    tgt_c = np.asarray(inputs["tgt_c"])
    src = np.asarray(inputs["src_e"], dtype=np.float32)
    spm = np.asarray(inputs["src_padding_mask"])
    type_emb = np.asarray(inputs["type_emb"], dtype=np.float32)
    W_ctype = np.asarray(inputs["W_ctype"], dtype=np.float32)
    b_ctype = np.asarray(inputs["b_ctype"], dtype=np.float32)
    W_obj = np.asarray(inputs["W_obj"], dtype=np.float32)
    b_obj = np.asarray(inputs["b_obj"], dtype=np.float32)
    W_dir = np.asarray(inputs["W_dir"], dtype=np.float32)
    b_dir = np.asarray(inputs["b_dir"], dtype=np.float32)

    # replicated weights, PE-friendly layouts
    W_p = np.concatenate([W_obj[:, :D], W_obj[:, 2 * D:3 * D]], axis=1)    # [1024, 2048]
    wmT = np.ascontiguousarray(
        W_p.T.reshape(16, 128, 8, 128).transpose(1, 2, 0, 3))              # [kk, m, k, mc]
    fold_main = np.concatenate(
        [type_emb @ W_obj[:, D:2 * D].T, b_obj[None, :]], axis=0).astype(np.float32)

    Wd_h, Wd_t, Wd_q, Wd_r = (W_dir[:, :D], W_dir[:, D:2 * D],
                              W_dir[:, 2 * D:3 * D], W_dir[:, 3 * D:])
    ws = np.zeros((24, 128, 8), np.float32)
    for blk, Wblk in enumerate([Wd_h, Wd_q, Wd_r]):
        ws[blk * 8:(blk + 1) * 8, :, 0:4] = Wblk.T.reshape(8, 128, 4)
    ws[0:8, :, 4:8] = W_ctype.T.reshape(8, 128, 4)
    wsT = np.ascontiguousarray(ws.transpose(1, 0, 2))                      # [128, 24, 8]
    fold_small = np.zeros((5, 8), np.float32)
    fold_small[0:4, 0:4] = type_emb @ Wd_t.T
    fold_small[4, 0:4] = b_dir
    fold_small[4, 4:8] = b_ctype

    ty_idx = tgt_c[:TH, :, 0].astype(np.int64)
    q_idx = tgt_c[:TH, :, 1].astype(np.int64)
    r_idx = tgt_c[:TH, :, 2].astype(np.int64)
    t_ar = np.arange(TH)

    in_maps = []
    for c in range(NCORES):
        bsl = slice(BPC * c, BPC * (c + 1))
        dec_c = dec[:TH, bsl, :]                              # [64, 16, D]
        headsM = dec_c.transpose(1, 0, 2).reshape(NTOK, D)    # token n = bl*64+t
        headsT = np.ascontiguousarray(
            headsM.T.reshape(8, 128, NG, NTG).transpose(1, 2, 0, 3))  # [p, g, ch, n']

        src_c = src[:, bsl, :]                                # [128(s), 16, D]
        src_nat = np.ascontiguousarray(src_c)
        srcT = np.ascontiguousarray(
            src_c.transpose(2, 1, 0).reshape(8, 128, BPC, S).transpose(1, 2, 0, 3))

        qr = np.zeros((128, BPC, 128), np.float32)
        for bl in range(BPC):
            bglob = BPC * c + bl
            qr[q_idx[:, bglob], bl, t_ar] = 1.0
            qr[r_idx[:, bglob], bl, TH + t_ar] = 1.0

        ty = np.zeros((5, NTOK), np.float32)
        tyc = ty_idx[:, bsl].T.reshape(-1)                    # n = bl*64+t
        ty[tyc, np.arange(NTOK)] = 1.0
        ty[4, :] = 1.0

        mask = np.where(spm[bsl, :], -np.inf, 0.0).astype(np.float32)[None]

        in_maps.append(dict(
            wmT=wmT, headsT=headsT, src_nat=src_nat, srcT=srcT, qr1h=qr,
            ty1h=ty, maskb=mask, fold_main=fold_main, wsT=wsT,
            fold_small=fold_small))
    return in_maps


def _simulate_core(im):
    """Numpy model of the device program (for decomposition checking)."""
    out_obj = np.zeros((TH, BPC, S), np.float32)
    out_small = np.zeros((8, NTOK), np.float32)
    for g in range(NG):
        objT = np.zeros((128, 24, NTG), np.float32)
        for ch in range(8):
            objT[:, ch, :] = im["headsT"][:, g, ch, :]
        for bl in range(GB):
            bg = g * GB + bl
            for ch in range(8):
                blob = im["src_nat"][:, bg, ch * 128:(ch + 1) * 128]   # [s, 128]
                res = blob.T @ im["qr1h"][:, bg, :]                    # [128, 128]
                objT[:, 8 + ch, bl * TH:(bl + 1) * TH] = res[:, 0:TH]
                objT[:, 16 + ch, bl * TH:(bl + 1) * TH] = res[:, TH:128]
        ptrT = np.zeros((128, 8, NTG), np.float32)
        for m in range(8):
            acc = im["fold_main"][:, m * 128:(m + 1) * 128].T @ im["ty1h"][:, g * NTG:(g + 1) * NTG]
            for k in range(16):
                acc = acc + im["wmT"][:, m, k, :].T @ objT[:, k, :]
            ptrT[:, m, :] = acc
        acc = im["fold_small"].T @ im["ty1h"][:, g * NTG:(g + 1) * NTG]
        for k in range(24):
            acc = acc + im["wsT"][:, k, :].T @ objT[:, k, :]
        out_small[:, g * NTG:(g + 1) * NTG] = acc
        for bl in range(GB):
            bg = g * GB + bl
            acc = np.broadcast_to(im["maskb"][0, bg, :][None, :], (TH, S)).copy()
            for ch in range(8):
                acc = acc + ptrT[:, ch, bl * TH:(bl + 1) * TH].T @ im["srcT"][:, bg, ch, :]
            out_obj[:, bg, :] = acc
    return dict(out_obj=out_obj, out_small=out_small)


def _assemble(results):
    objs = np.concatenate([results[c]["out_obj"] for c in range(NCORES)], axis=1)
    object_selections = np.ascontiguousarray(objs.reshape(TH * B, S))
    sm = np.stack([results[c]["out_small"] for c in range(NCORES)])   # [core, 8, 1024]
    sm = sm.reshape(NCORES, 8, BPC, TH).transpose(3, 0, 2, 1)         # [t, core, bl, row]
    flat = np.ascontiguousarray(sm.reshape(TH * B, 8))
    direction_selections = np.ascontiguousarray(flat[:, 0:4])
    type_selections = np.ascontiguousarray(flat[:, 4:8])
    return type_selections, object_selections, direction_selections


def kernel_sim(**inputs):
    """Pure-numpy model of the full pipeline (no device)."""
    in_maps = _prep_in_maps(inputs)
    results = [_simulate_core(im) for im in in_maps]
    return _assemble(results)


def kernel(**inputs):
    from concourse.bass_utils import run_bass_kernel_spmd
    nc = _get_nc()
    in_maps = _prep_in_maps(inputs)
    res = run_bass_kernel_spmd(nc, in_maps, core_ids=list(range(NCORES)))
    return _assemble(res.results)
